# revision 6
# baseline (speedup 1.0000x reference)
"""DySample (B=16,C=64,H=W=128, scale=2, groups=4) Trainium2 kernel — v3.

Fixed 4-tap stencil with data-dependent weights (derivation verified vs
reference). Per output quadrant (dy,dx) of group g, k = dx (g even) / dy:
  k=1: out = V + wx*HD + wy*VD + wx*wy*XD     (forward diffs)
  k=0: out = V + wx*HDm + wy*VDdn + wx*wy*XDL (backward diffs, signs folded
       into the host-side conv weight/bias preparation)

Measured-cost driven design (all bf16):
 - DVE bf16 tensor_tensor runs 2x (~1.2us per [128,2048]) even with
   flat-shifted (x+1/x-1) reads and broadcast weight APs -> all 6 diff
   tensors are plain DVE TTs on contiguous tiles; edge columns are exact
   zeros fixed by one small memset each.
 - Cross-partition shifts (Vup/Vdn) via cheap SBUF->SBUF row-shifted DMAs.
 - Per quadrant: DVE does 3 weight mults + 1 add; TensorEngine merges
   V + (m1+m2) + mc in PSUM via identity matmuls (contiguous rhs only);
   ScalarEngine drains the PSUM quadrant straight into the interleaved
   output buffer (the only 1x-strided op, on the otherwise-idle engine).
 - GpSimd does only tiny memsets (big gpsimd TTs are slow AND poison
   concurrent DVE via shared SBUF ports).
 - Dedicated DMA queues: loads on sync, output stores issued by scalar
   right after its own drains (no semaphore stalls blocking loads).
 - 1x1 conv (PE matmul, block-diag weights for both images) emits the
   folded weight planes; o-major -> y-major relayout via 2MiB DRAM bounce.
"""
import sys, types, ctypes, contextlib

sys.path.insert(0, "/opt/trn_rl_repo")

import numpy as np

_SO_PATH = "/opt/axon/libaxon_pjrt.so"


def _install_hooks():
    if "antenv.axon_hooks" in sys.modules:
        return
    mod = types.ModuleType("antenv.axon_hooks")
    mod._hook = None
    mod.set_axon_ntff_profile_hook = lambda h: setattr(mod, "_hook", h)
    mod.get_axon_ntff_profile_hook = lambda: mod._hook
    sys.modules["antenv.axon_hooks"] = mod
    try:
        lib = ctypes.CDLL(_SO_PATH)
        if not hasattr(lib, "axon_start_nrt_profile"):
            return
        lib.axon_start_nrt_profile.argtypes = [ctypes.POINTER(ctypes.c_int64), ctypes.c_size_t]
        lib.axon_start_nrt_profile.restype = ctypes.c_int64
        lib.axon_stop_nrt_profile.argtypes = [ctypes.c_char_p]
        lib.axon_stop_nrt_profile.restype = ctypes.c_int64

        @contextlib.contextmanager
        def _hook(output_dir, device_ids):
            import jax
            jax.devices()
            if device_ids:
                ids = (ctypes.c_int64 * len(device_ids))(*device_ids)
                rc = lib.axon_start_nrt_profile(ids, len(device_ids))
            else:
                rc = lib.axon_start_nrt_profile(None, 0)
            if rc != 0:
                raise RuntimeError(f"axon_start_nrt_profile rc={rc}")
            try:
                yield
            finally:
                lib.axon_stop_nrt_profile(str(output_dir).encode())

        mod.set_axon_ntff_profile_hook(_hook)
    except OSError:
        pass


_install_hooks()

import concourse.bass as bass
import concourse.bacc as bacc
import concourse.tile as tile
import concourse.mybir as mybir
from contextlib import ExitStack
from concourse.bass_utils import run_bass_kernel_spmd

f32 = mybir.dt.float32
bf16 = mybir.dt.bfloat16
Op = mybir.AluOpType

N_CORES = 8
B, C, H, W = 16, 64, 128, 128
BPC = B // N_CORES
G, S = 4, 2
NO = 32
CB = 16
FD = CB * W          # 2048
PADF = 16            # front/back spacer elems on shift-read tiles
FT = FD + 2 * PADF   # 2080

_cache = {}


def _build():
    nc = bacc.Bacc("TRN2", target_bir_lowering=False, debug=False, num_devices=1)
    xb_ap = nc.dram_tensor("xb", [BPC * C, H * W], bf16, kind="ExternalInput").ap()
    xp_ap = nc.dram_tensor("xp", [BPC * G, H, FD], bf16, kind="ExternalInput").ap()
    wg_ap = nc.dram_tensor("wg", [129, C], bf16, kind="ExternalInput").ap()
    sm_ap = nc.dram_tensor("sm", [128, 128], bf16, kind="ExternalInput").ap()
    out_ap = nc.dram_tensor("out", [BPC, C, 2 * H, 2 * W], bf16, kind="ExternalOutput").ap()
    scr_ap = nc.dram_tensor("scr", [C, H * W], bf16, kind="Internal").ap()

    with tile.TileContext(nc) as tc, ExitStack() as ctx:
        pool = ctx.enter_context(tc.tile_pool(name="p", bufs=1))
        pool2 = ctx.enter_context(tc.tile_pool(name="p2", bufs=2))
        pool3 = ctx.enter_context(tc.tile_pool(name="p3", bufs=2))

        # ---------- constants ----------
        smat = pool.tile([128, 128], bf16, tag="smat")
        nc.sync.dma_start(smat[:], sm_ap[:])
        SI = smat[:, 0:128]  # identity (for PSUM accumulation matmuls)

        waug = pool.tile([128, C], bf16, tag="waug")
        nc.sync.dma_start(waug[:], wg_ap[0:128, :])
        brow = pool.tile([1, C], bf16, tag="brow")
        nc.sync.dma_start(brow[:], wg_ap[128:129, :])
        ones = pool.tile([1, 512], bf16, tag="ones")
        nc.vector.memset(ones[:], 1.0)

        wsb = pool.tile([C, H * W], bf16, tag="wsb")
        off_y = pool.tile([128, C * W], bf16, tag="offy")     # [y,(img o32,x)]
        wxy = pool.tile([128, BPC * CB * W], bf16, tag="wxy")  # [y,(img,o16,x)]

        # ---------- conv: folded weight planes (o-major) ----------
        with tc.tile_pool(name="pc", bufs=2, space="PSUM") as ppc:
            for h4 in range(8):
                xc = pool2.tile([128, 2048], bf16, tag="xc")
                nc.sync.dma_start(xc[:], xb_ap[:, bass.ts(h4, 2048)])
                for q in range(4):
                    ps = ppc.tile([C, 512], f32, tag="cps")
                    nc.tensor.matmul(ps[:], waug[:], xc[:, bass.ts(q, 512)],
                                     start=True, stop=False)
                    nc.tensor.matmul(ps[:], brow[:], ones[:],
                                     start=False, stop=True)
                    nc.scalar.copy(wsb[:, bass.ts(h4 * 4 + q, 512)], ps[:])

        # o-major -> y-major via DRAM bounce (2 MiB each way)
        nc.sync.dma_start(scr_ap[:], wsb[:])
        nc.sync.dma_start(
            off_y[:].rearrange("y (i x) -> y i x", i=C),
            scr_ap[:].rearrange("i (y x) -> y i x", x=W))
        for b in range(BPC):
            nc.vector.tensor_tensor(
                wxy[:, bass.ts(b, 2048)],
                off_y[:, b * 4096: b * 4096 + 2048],
                off_y[:, b * 4096 + 2048: b * 4096 + 4096], Op.mult)

        # ---------- stencil ----------
        with tc.tile_pool(name="pp", bufs=2, space="PSUM") as pps:
            for b in range(BPC):
                for g in range(G):
                    # V (contiguous, with shift-read spacers)
                    Vc = pool2.tile([128, FT], bf16, tag="Vc")
                    V0 = Vc[:, PADF:PADF + FD]
                    nc.sync.dma_start(V0, xp_ap[b * G + g])
                    # row-shifted copies (SBUF->SBUF DMA; edge rows dup'd)
                    Vup = pool2.tile([128, FD], bf16, tag="Vup")
                    Vdn = pool2.tile([128, FD], bf16, tag="Vdn")
                    nc.sync.dma_start(Vup[0:127, :], V0[1:128])
                    nc.sync.dma_start(Vup[127:128, :], V0[127:128])
                    nc.sync.dma_start(Vdn[1:128, :], V0[0:127])
                    nc.sync.dma_start(Vdn[0:1, :], V0[0:1])

                    # 6 diff taps, all DVE 2x TTs on contiguous tiles
                    VD = pool3.tile([128, FT], bf16, tag="VD")
                    VDdn = pool3.tile([128, FT], bf16, tag="VDdn")
                    HD = pool3.tile([128, FD], bf16, tag="HD")
                    HDm = pool3.tile([128, FD], bf16, tag="HDm")
                    XD = pool3.tile([128, FD], bf16, tag="XD")
                    XDL = pool3.tile([128, FD], bf16, tag="XDL")
                    VD0 = VD[:, PADF:PADF + FD]
                    VDdn0 = VDdn[:, PADF:PADF + FD]
                    nc.vector.tensor_tensor(VD0, Vup[:], V0, Op.subtract)
                    nc.vector.tensor_tensor(VDdn0, V0, Vdn[:], Op.subtract)
                    nc.vector.tensor_tensor(HD[:], Vc[:, PADF + 1:PADF + 1 + FD], V0, Op.subtract)
                    nc.vector.tensor_tensor(HDm[:], V0, Vc[:, PADF - 1:PADF - 1 + FD], Op.subtract)
                    nc.vector.tensor_tensor(XD[:], VD[:, PADF + 1:PADF + 1 + FD], VD0, Op.subtract)
                    nc.vector.tensor_tensor(XDL[:], VDdn0, VDdn[:, PADF - 1:PADF - 1 + FD], Op.subtract)
                    # exact zeros at clamped edge columns (kill c-block leakage)
                    for t in (HD, XD):
                        nc.gpsimd.memset(
                            t[:].rearrange("y (c x) -> y c x", x=W)[:, :, 127:128], 0.0)
                    for t in (HDm, XDL):
                        nc.gpsimd.memset(
                            t[:].rearrange("y (c x) -> y c x", x=W)[:, :, 0:1], 0.0)

                    tHD = HD[:].rearrange("y (c x) -> y c x", x=W)
                    tHDm = HDm[:].rearrange("y (c x) -> y c x", x=W)
                    tVD = VD0.rearrange("y (c x) -> y c x", x=W)
                    tVDdn = VDdn0.rearrange("y (c x) -> y c x", x=W)
                    tXD = XD[:].rearrange("y (c x) -> y c x", x=W)
                    tXDL = XDL[:].rearrange("y (c x) -> y c x", x=W)

                    AS2 = pool2.tile([128, CB * 2 * 2 * W], bf16, tag="AS2")
                    ASv = AS2[:].rearrange("y (c d x two) -> y c d x two",
                                           c=CB, d=2, two=2)

                    for dy in range(2):
                        for dx in range(2):
                            o = g * 4 + dy * 2 + dx
                            k = dx if g % 2 == 0 else dy
                            col = (b * 32 + o) * W
                            wxb = off_y[:, col:col + W].unsqueeze(1).broadcast_to([128, CB, W])
                            wyb = off_y[:, col + 16 * W:col + 17 * W].unsqueeze(1).broadcast_to([128, CB, W])
                            wxyb = wxy[:, (b * 16 + o) * W:(b * 16 + o + 1) * W].unsqueeze(1).broadcast_to([128, CB, W])
                            if k == 1:
                                tx, ty, tcr = tHD, tVD, tXD
                            else:
                                tx, ty, tcr = tHDm, tVDdn, tXDL

                            m1 = pool3.tile([128, FD], bf16, tag="m1")
                            m2 = pool3.tile([128, FD], bf16, tag="m2")
                            mc = pool3.tile([128, FD], bf16, tag="mc")
                            m1v = m1[:].rearrange("y (c x) -> y c x", x=W)
                            m2v = m2[:].rearrange("y (c x) -> y c x", x=W)
                            mcv = mc[:].rearrange("y (c x) -> y c x", x=W)
                            nc.vector.tensor_tensor(m1v, tx, wxb, Op.mult)
                            nc.vector.tensor_tensor(m2v, ty, wyb, Op.mult)
                            nc.vector.tensor_tensor(mcv, tcr, wxyb, Op.mult)
                            nc.vector.tensor_tensor(m1[:], m1[:], m2[:], Op.add)

                            # PSUM: V + s12 + mc via identity matmuls
                            qp = pps.tile([128, 2048], f32, tag="ps")
                            for cc in range(4):
                                nc.tensor.matmul(qp[:, bass.ts(cc, 512)], SI,
                                                 V0[:, bass.ts(cc, 512)],
                                                 start=True, stop=False)
                            for cc in range(4):
                                nc.tensor.matmul(qp[:, bass.ts(cc, 512)], SI,
                                                 m1[:, bass.ts(cc, 512)],
                                                 start=False, stop=False)
                            for cc in range(4):
                                nc.tensor.matmul(qp[:, bass.ts(cc, 512)], SI,
                                                 mc[:, bass.ts(cc, 512)],
                                                 start=False, stop=True)
                            nc.scalar.copy(
                                ASv[:, :, dy, :, dx],
                                qp[:].rearrange("y (c x) -> y c x", x=W))

                    # out store issued by scalar right after its own drains
                    nc.scalar.dma_start(
                        out_ap[b, g * CB:(g + 1) * CB].rearrange(
                            "c (y d) x -> y c d x", d=2),
                        AS2[:].rearrange("y (c d x) -> y c d x", c=CB, d=2))

    nc.compile()
    return nc


def _host_prep(x, w_off, b_off):
    import ml_dtypes
    nbf = ml_dtypes.bfloat16
    x = np.asarray(x, dtype=np.float32)

    w = 0.25 * np.asarray(w_off, dtype=np.float32)
    bb = 0.25 * np.asarray(b_off, dtype=np.float32)
    bf = bb.copy()
    for o in range(16):
        g, r = divmod(o, 4)
        dy, dx = divmod(r, 2)
        k = dx if g % 2 == 0 else dy
        sgn = 1.0 if k == 1 else -1.0
        bf[o] = bb[o] + sgn * 0.25
        bf[16 + o] = bb[16 + o] + sgn * 0.25
    waug = np.zeros((128, 64), dtype=np.float32)
    waug[0:64, 0:32] = w.T
    waug[64:128, 32:64] = w.T
    brow = np.concatenate([bf, bf])[None, :]
    wg = np.concatenate([waug, brow], axis=0).astype(nbf)

    sm = np.eye(128, dtype=np.float32).astype(nbf)

    xbf = x.astype(nbf)
    xpre = np.ascontiguousarray(
        xbf.reshape(B, G, CB, H, W).transpose(0, 1, 3, 2, 4).reshape(B, G, H, CB * W))
    xbc = np.ascontiguousarray(xbf.reshape(B, C, H * W))
    return xbc, xpre, wg, sm


def kernel(x, w_off, b_off):
    key = "k"
    if key not in _cache:
        _cache[key] = _build()
    nc = _cache[key]

    xbc, xpre, wg, sm = _host_prep(x, w_off, b_off)
    in_maps = []
    for i in range(N_CORES):
        xb = xbc[BPC * i:BPC * (i + 1)].reshape(BPC * C, H * W)
        xp = xpre[BPC * i:BPC * (i + 1)].reshape(BPC * G, H, CB * W)
        in_maps.append({"xb": np.ascontiguousarray(xb),
                        "xp": np.ascontiguousarray(xp),
                        "wg": wg, "sm": sm})

    res = run_bass_kernel_spmd(nc, in_maps, core_ids=list(range(N_CORES)))
    out = np.empty((B, C, 2 * H, 2 * W), dtype=np.float32)
    for i in range(N_CORES):
        out[BPC * i:BPC * (i + 1)] = np.asarray(
            res.results[i]["out"], dtype=np.float32)
    return out


# revision 13
# speedup vs baseline: 1.7365x; 1.7365x over previous
"""DySample (B=16,C=64,H=W=128, scale=2, groups=4) Trainium2 kernel — v3.

Fixed 4-tap stencil with data-dependent weights (derivation verified vs
reference). Per output quadrant (dy,dx) of group g, k = dx (g even) / dy:
  k=1: out = V + wx*HD + wy*VD + wx*wy*XD     (forward diffs)
  k=0: out = V + wx*HDm + wy*VDdn + wx*wy*XDL (backward diffs, signs folded
       into the host-side conv weight/bias preparation)

Measured-cost driven design (all bf16):
 - DVE bf16 tensor_tensor runs 2x (~1.2us per [128,2048]) even with
   flat-shifted (x+1/x-1) reads and broadcast weight APs -> all 6 diff
   tensors are plain DVE TTs on contiguous tiles; edge columns are exact
   zeros fixed by one small memset each.
 - Cross-partition shifts (Vup/Vdn) via cheap SBUF->SBUF row-shifted DMAs.
 - Per quadrant: DVE does 3 weight mults + 1 add; TensorEngine merges
   V + (m1+m2) + mc in PSUM via identity matmuls (contiguous rhs only);
   ScalarEngine drains the PSUM quadrant straight into the interleaved
   output buffer (the only 1x-strided op, on the otherwise-idle engine).
 - GpSimd does only tiny memsets (big gpsimd TTs are slow AND poison
   concurrent DVE via shared SBUF ports).
 - Dedicated DMA queues: loads on sync, output stores issued by scalar
   right after its own drains (no semaphore stalls blocking loads).
 - 1x1 conv (PE matmul, block-diag weights for both images) emits the
   folded weight planes; o-major -> y-major relayout via 2MiB DRAM bounce.
"""
import sys, types, ctypes, contextlib

sys.path.insert(0, "/opt/trn_rl_repo")

import numpy as np

_SO_PATH = "/opt/axon/libaxon_pjrt.so"


def _install_hooks():
    if "antenv.axon_hooks" in sys.modules:
        return
    mod = types.ModuleType("antenv.axon_hooks")
    mod._hook = None
    mod.set_axon_ntff_profile_hook = lambda h: setattr(mod, "_hook", h)
    mod.get_axon_ntff_profile_hook = lambda: mod._hook
    sys.modules["antenv.axon_hooks"] = mod
    try:
        lib = ctypes.CDLL(_SO_PATH)
        if not hasattr(lib, "axon_start_nrt_profile"):
            return
        lib.axon_start_nrt_profile.argtypes = [ctypes.POINTER(ctypes.c_int64), ctypes.c_size_t]
        lib.axon_start_nrt_profile.restype = ctypes.c_int64
        lib.axon_stop_nrt_profile.argtypes = [ctypes.c_char_p]
        lib.axon_stop_nrt_profile.restype = ctypes.c_int64

        @contextlib.contextmanager
        def _hook(output_dir, device_ids):
            import jax
            jax.devices()
            if device_ids:
                ids = (ctypes.c_int64 * len(device_ids))(*device_ids)
                rc = lib.axon_start_nrt_profile(ids, len(device_ids))
            else:
                rc = lib.axon_start_nrt_profile(None, 0)
            if rc != 0:
                raise RuntimeError(f"axon_start_nrt_profile rc={rc}")
            try:
                yield
            finally:
                lib.axon_stop_nrt_profile(str(output_dir).encode())

        mod.set_axon_ntff_profile_hook(_hook)
    except OSError:
        pass


_install_hooks()

import concourse.bass as bass
import concourse.bacc as bacc
import concourse.tile as tile
import concourse.mybir as mybir
from contextlib import ExitStack
from concourse.bass_utils import run_bass_kernel_spmd

f32 = mybir.dt.float32
bf16 = mybir.dt.bfloat16
Op = mybir.AluOpType

N_CORES = 8
B, C, H, W = 16, 64, 128, 128
BPC = B // N_CORES
G, S = 4, 2
NO = 32
CB = 16
FD = CB * W          # 2048
PADF = 16            # front/back spacer elems on shift-read tiles
FT = FD + 2 * PADF   # 2080

_cache = {}


def _build():
    nc = bacc.Bacc("TRN2", target_bir_lowering=False, debug=False, num_devices=1)
    xb_ap = nc.dram_tensor("xb", [BPC * C, H * W], bf16, kind="ExternalInput").ap()
    # xp rows: [0]=dup(y0), [1..128]=y0..y127, [129]=dup(y127)  (clamp rows)
    xp_ap = nc.dram_tensor("xp", [BPC * G, H + 2, FD], bf16, kind="ExternalInput").ap()
    wg_ap = nc.dram_tensor("wg", [128, C], bf16, kind="ExternalInput").ap()
    sm_ap = nc.dram_tensor("sm", [128, 129], bf16, kind="ExternalInput").ap()
    out_ap = nc.dram_tensor("out", [BPC, C, 2 * H, 2 * W], bf16, kind="ExternalOutput").ap()
    scr_ap = nc.dram_tensor("scr", [C, H * W], bf16, kind="Internal").ap()

    with tile.TileContext(nc) as tc, ExitStack() as ctx:
        pool = ctx.enter_context(tc.tile_pool(name="p", bufs=1))
        pool2 = ctx.enter_context(tc.tile_pool(name="p2", bufs=2))
        pool3 = ctx.enter_context(tc.tile_pool(name="p3", bufs=2))

        # ---------- constants ----------
        smat = pool.tile([128, 129], bf16, tag="smat")
        nc.sync.dma_start(smat[:], sm_ap[:])
        SI = smat[:, 0:128]           # identity (for PSUM accumulation matmuls)
        bias = smat[0:C, 128:129]     # folded conv bias, per o-partition

        waug = pool.tile([128, C], bf16, tag="waug")
        nc.sync.dma_start(waug[:], wg_ap[:])

        wsb = pool.tile([C, H * W], bf16, tag="wsb")
        off_y = pool.tile([128, C * W], bf16, tag="offy")     # [y,(img o32,x)]
        wxy = pool.tile([128, BPC * CB * W], bf16, tag="wxy")  # [y,(img,o16,x)]

        # ---------- conv: folded weight planes (o-major) ----------
        with tc.tile_pool(name="pc", bufs=2, space="PSUM") as ppc:
            for h4 in range(8):
                xc = pool2.tile([128, 2048], bf16, tag="xc")
                nc.sync.dma_start(xc[:], xb_ap[:, bass.ts(h4, 2048)])
                for q in range(4):
                    ps = ppc.tile([C, 512], f32, tag="cps")
                    nc.tensor.matmul(ps[:], waug[:], xc[:, bass.ts(q, 512)],
                                     start=True, stop=True)
                    nc.scalar.activation(
                        wsb[:, bass.ts(h4 * 4 + q, 512)], ps[:],
                        mybir.ActivationFunctionType.Identity, bias=bias)

        # o-major -> y-major via DRAM bounce (2 MiB each way)
        nc.sync.dma_start(scr_ap[:], wsb[:])
        nc.sync.dma_start(
            off_y[:].rearrange("y (i x) -> y i x", i=C),
            scr_ap[:].rearrange("i (y x) -> y i x", x=W))
        for b in range(BPC):
            nc.vector.tensor_tensor(
                wxy[:, bass.ts(b, 2048)],
                off_y[:, b * 4096: b * 4096 + 2048],
                off_y[:, b * 4096 + 2048: b * 4096 + 4096], Op.mult)

        # ---------- stencil ----------
        with tc.tile_pool(name="pp", bufs=2, space="PSUM") as pps:
            for b in range(BPC):
                for g in range(G):
                    # V + row-shifted variants: three clean 128-row HBM loads
                    # (xp has host-side clamp rows at 0 and 129)
                    Vc = pool2.tile([128, FT], bf16, tag="Vc")
                    V0 = Vc[:, PADF:PADF + FD]
                    nc.sync.dma_start(V0, xp_ap[b * G + g, 1:129])
                    Vup = pool2.tile([128, FD], bf16, tag="Vup")
                    Vdn = pool2.tile([128, FD], bf16, tag="Vdn")
                    nc.sync.dma_start(Vup[:], xp_ap[b * G + g, 2:130])
                    nc.sync.dma_start(Vdn[:], xp_ap[b * G + g, 0:128])

                    # 6 diff taps, all DVE 2x TTs on contiguous tiles
                    VD = pool3.tile([128, FT], bf16, tag="VD")
                    VDdn = pool3.tile([128, FT], bf16, tag="VDdn")
                    HD = pool3.tile([128, FD], bf16, tag="HD")
                    HDm = pool3.tile([128, FD], bf16, tag="HDm")
                    XD = pool3.tile([128, FD], bf16, tag="XD")
                    XDL = pool3.tile([128, FD], bf16, tag="XDL")
                    VD0 = VD[:, PADF:PADF + FD]
                    VDdn0 = VDdn[:, PADF:PADF + FD]
                    nc.vector.tensor_tensor(VD0, Vup[:], V0, Op.subtract)
                    nc.vector.tensor_tensor(VDdn0, V0, Vdn[:], Op.subtract)
                    nc.vector.tensor_tensor(HD[:], Vc[:, PADF + 1:PADF + 1 + FD], V0, Op.subtract)
                    nc.vector.tensor_tensor(HDm[:], V0, Vc[:, PADF - 1:PADF - 1 + FD], Op.subtract)
                    nc.vector.tensor_tensor(XD[:], VD[:, PADF + 1:PADF + 1 + FD], VD0, Op.subtract)
                    nc.vector.tensor_tensor(XDL[:], VDdn0, VDdn[:, PADF - 1:PADF - 1 + FD], Op.subtract)
                    # exact zeros at clamped edge columns (kill c-block leakage)
                    for t in (HD, XD):
                        nc.gpsimd.memset(
                            t[:].rearrange("y (c x) -> y c x", x=W)[:, :, 127:128], 0.0)
                    for t in (HDm, XDL):
                        nc.gpsimd.memset(
                            t[:].rearrange("y (c x) -> y c x", x=W)[:, :, 0:1], 0.0)

                    tHD = HD[:].rearrange("y (c x) -> y c x", x=W)
                    tHDm = HDm[:].rearrange("y (c x) -> y c x", x=W)
                    tVD = VD0.rearrange("y (c x) -> y c x", x=W)
                    tVDdn = VDdn0.rearrange("y (c x) -> y c x", x=W)
                    tXD = XD[:].rearrange("y (c x) -> y c x", x=W)
                    tXDL = XDL[:].rearrange("y (c x) -> y c x", x=W)

                    AS2 = pool2.tile([128, CB * 2 * 2 * W], bf16, tag="AS2")
                    ASv = AS2[:].rearrange("y (c d x two) -> y c d x two",
                                           c=CB, d=2, two=2)

                    for dy in range(2):
                        for dx in range(2):
                            o = g * 4 + dy * 2 + dx
                            k = dx if g % 2 == 0 else dy
                            col = (b * 32 + o) * W
                            wxb = off_y[:, col:col + W].unsqueeze(1).broadcast_to([128, CB, W])
                            wyb = off_y[:, col + 16 * W:col + 17 * W].unsqueeze(1).broadcast_to([128, CB, W])
                            wxyb = wxy[:, (b * 16 + o) * W:(b * 16 + o + 1) * W].unsqueeze(1).broadcast_to([128, CB, W])
                            if k == 1:
                                tx, ty, tcr = tHD, tVD, tXD
                            else:
                                tx, ty, tcr = tHDm, tVDdn, tXDL

                            m1 = pool3.tile([128, FD], bf16, tag="m1")
                            m2 = pool3.tile([128, FD], bf16, tag="m2")
                            mc = pool3.tile([128, FD], bf16, tag="mc")
                            m1v = m1[:].rearrange("y (c x) -> y c x", x=W)
                            m2v = m2[:].rearrange("y (c x) -> y c x", x=W)
                            mcv = mc[:].rearrange("y (c x) -> y c x", x=W)
                            nc.vector.tensor_tensor(m1v, tx, wxb, Op.mult)
                            nc.vector.tensor_tensor(m2v, ty, wyb, Op.mult)
                            nc.vector.tensor_tensor(mcv, tcr, wxyb, Op.mult)
                            nc.vector.tensor_tensor(m1[:], m1[:], m2[:], Op.add)

                            # PSUM: V + s12 + mc via identity matmuls
                            qp = pps.tile([128, 2048], f32, tag="ps")
                            for cc in range(4):
                                nc.tensor.matmul(qp[:, bass.ts(cc, 512)], SI,
                                                 V0[:, bass.ts(cc, 512)],
                                                 start=True, stop=False)
                            for cc in range(4):
                                nc.tensor.matmul(qp[:, bass.ts(cc, 512)], SI,
                                                 m1[:, bass.ts(cc, 512)],
                                                 start=False, stop=False)
                            for cc in range(4):
                                nc.tensor.matmul(qp[:, bass.ts(cc, 512)], SI,
                                                 mc[:, bass.ts(cc, 512)],
                                                 start=False, stop=True)
                            nc.scalar.copy(
                                ASv[:, :, dy, :, dx],
                                qp[:].rearrange("y (c x) -> y c x", x=W))

                    # out store issued by scalar right after its own drains
                    nc.scalar.dma_start(
                        out_ap[b, g * CB:(g + 1) * CB].rearrange(
                            "c (y d) x -> y c d x", d=2),
                        AS2[:].rearrange("y (c d x) -> y c d x", c=CB, d=2))

    nc.compile()
    return nc


def _host_prep(x, w_off, b_off):
    import ml_dtypes
    nbf = ml_dtypes.bfloat16
    x = np.asarray(x, dtype=np.float32)

    w = 0.25 * np.asarray(w_off, dtype=np.float32)
    bb = 0.25 * np.asarray(b_off, dtype=np.float32)
    bf = bb.copy()
    for o in range(16):
        g, r = divmod(o, 4)
        dy, dx = divmod(r, 2)
        k = dx if g % 2 == 0 else dy
        sgn = 1.0 if k == 1 else -1.0
        bf[o] = bb[o] + sgn * 0.25
        bf[16 + o] = bb[16 + o] + sgn * 0.25
    waug = np.zeros((128, 64), dtype=np.float32)
    waug[0:64, 0:32] = w.T
    waug[64:128, 32:64] = w.T
    wg = waug.astype(nbf)

    sm = np.zeros((128, 129), dtype=np.float32)
    sm[:, 0:128] = np.eye(128, dtype=np.float32)
    sm[0:64, 128] = np.concatenate([bf, bf])
    sm = sm.astype(nbf)

    xbf = x.astype(nbf)
    xg = xbf.reshape(B, G, CB, H, W).transpose(0, 1, 3, 2, 4)  # [B,G,H,CB,W]
    xpre = np.empty((B, G, H + 2, CB, W), dtype=nbf)
    xpre[:, :, 1:H + 1] = xg
    xpre[:, :, 0] = xg[:, :, 0]
    xpre[:, :, H + 1] = xg[:, :, H - 1]
    xpre = np.ascontiguousarray(xpre.reshape(B, G, H + 2, CB * W))
    xbc = np.ascontiguousarray(xbf.reshape(B, C, H * W))
    return xbc, xpre, wg, sm


def kernel(x, w_off, b_off):
    key = "k"
    if key not in _cache:
        _cache[key] = _build()
    nc = _cache[key]

    xbc, xpre, wg, sm = _host_prep(x, w_off, b_off)
    in_maps = []
    for i in range(N_CORES):
        xb = xbc[BPC * i:BPC * (i + 1)].reshape(BPC * C, H * W)
        xp = xpre[BPC * i:BPC * (i + 1)].reshape(BPC * G, H + 2, CB * W)
        in_maps.append({"xb": np.ascontiguousarray(xb),
                        "xp": np.ascontiguousarray(xp),
                        "wg": wg, "sm": sm})

    res = run_bass_kernel_spmd(nc, in_maps, core_ids=list(range(N_CORES)))
    out = np.empty((B, C, 2 * H, 2 * W), dtype=np.float32)
    for i in range(N_CORES):
        out[BPC * i:BPC * (i + 1)] = np.asarray(
            res.results[i]["out"], dtype=np.float32)
    return out
